# revision 11
# baseline (speedup 1.0000x reference)
"""Trainium2 Bass kernel for nn_Attention_24404004176269.

Rotary causal attention with per-head inputs/weights:
  x_{q,k,v}: [B=2, S=2048, H=12, M=768], W_{Q,K,V}: [H, 768, 64], W_O: [H, 64, 768]
  out[b,s,h,:] = softmax(causal(rot(q) rot(k)^T / 8)) @ v @ W_O[h] (+ biases)

Sharding: the 24 (b, h) pairs are fully independent -> 3 pairs per core on 8 cores.

Per-core plan (all compute in bf16 with fp32 PSUM accumulation):
  - host casts inputs to bf16 and pre-chunks x as [pair, mc, S, 128] so the
    xbar DMA-transpose loads xT tiles [128m, S] straight from DRAM
  - qT/kT [64, S] = W^T @ xT, rotary (+ bias, + 1/sqrt(8) scale folded into
    the cos/sin tables) applied during PSUM eviction
  - v [S, 64] = xT^T @ W_V, stored as [128k, 65] tiles with a ones column so
    the z matmul also produces softmax row-sums
  - scores computed transposed: sT [128k, 512q] = kT_blk^T @ qT_chunk, so
    exp(sT) is directly the rhs of zT [65, 512q] += v_blk^T-style accumulation
  - causal masking: fully-masked key blocks are skipped; diagonal blocks get a
    0/1 multiplicative mask after exp (exact, matches exp(-1e5-max)==0)
  - normalization 1/rowsum broadcast across the 64 d-partitions via a K=1
    matmul, multiplied into zT during eviction
  - out [128q, 768] = zT_blk^T @ W_O, evicted bf16, DMA'd out
  - b_V and b_O are folded in exactly on the host: softmax rows sum to 1, so
    z = P(v + b_V) = Pv + b_V, giving out += b_V @ W_O + b_O/H per head.
"""

import sys

for _p in ("/opt/trn_rl_repo",):
    if _p not in sys.path:
        sys.path.insert(0, _p)

import contextlib

import ml_dtypes
import numpy as np

import concourse.bass as bass
import concourse.tile as tile
from concourse import bacc, mybir
from concourse.bass_utils import run_bass_kernel_spmd

B, S, H, M, DH = 2, 2048, 12, 768, 64
N_CORES = 8
PAIRS = (B * H) // N_CORES  # 3 (b, h) pairs per core
MC = M // 128  # 6 contraction chunks
QC = 4  # q chunks of 512
QCHUNK = 512
ROTARY_BASE = 10000.0
GS = float(np.sqrt(1.0 / np.sqrt(float(DH))))  # sqrt(1/8), folded into q AND k

BF16 = mybir.dt.bfloat16
F32 = mybir.dt.float32
MUL = mybir.AluOpType.mult
ADD = mybir.AluOpType.add
EXP = mybir.ActivationFunctionType.Exp

TRACE = False  # test.py can flip this for neuron-profile timing


def build_program():
    """Build the per-core Bass program (identical on all cores, SPMD by data)."""
    nc = bacc.Bacc(None, target_bir_lowering=False, debug=False, num_devices=N_CORES)

    dram = {}
    for t in ("xq", "xk", "xv"):
        dram[t] = nc.dram_tensor(t, [PAIRS, MC, S, 128], BF16, kind="ExternalInput").ap()
    for t in ("wq", "wk", "wv"):
        # host pre-packed: [128, PAIRS*MC*DH], column block (p*MC+mc)*DH is
        # W[head_p][mc*128:(mc+1)*128, :]
        dram[t] = nc.dram_tensor(t, [128, PAIRS * MC * DH], BF16, kind="ExternalInput").ap()
    dram["wo"] = nc.dram_tensor("wo", [DH, PAIRS * M], BF16, kind="ExternalInput").ap()
    dram["cosc"] = nc.dram_tensor("cosc", [DH, S], F32, kind="ExternalInput").ap()
    dram["sinc"] = nc.dram_tensor("sinc", [DH, S], F32, kind="ExternalInput").ap()
    dram["maskt"] = nc.dram_tensor("maskt", [128, 1024], BF16, kind="ExternalInput").ap()
    dram["bq"] = nc.dram_tensor("bq", [DH, PAIRS], F32, kind="ExternalInput").ap()
    dram["bk"] = nc.dram_tensor("bk", [DH, PAIRS], F32, kind="ExternalInput").ap()
    # partition-flipped copies (rows 0:32 <-> 32:64) so the rotary flip term's
    # scalar operand shares its base partition with the sin table slice
    dram["bqf"] = nc.dram_tensor("bqf", [DH, PAIRS], F32, kind="ExternalInput").ap()
    dram["bkf"] = nc.dram_tensor("bkf", [DH, PAIRS], F32, kind="ExternalInput").ap()
    out_d = nc.dram_tensor("out", [PAIRS, S, M], BF16, kind="ExternalOutput").ap()

    with tile.TileContext(nc) as tc, contextlib.ExitStack() as ctx:
        ep = ctx.enter_context

        const = ep(tc.tile_pool(name="const", bufs=1))
        xt_pool = ep(tc.tile_pool(name="xt", bufs=24))
        qk_pool = ep(tc.tile_pool(name="qk", bufs=4))
        vv_pool = ep(tc.tile_pool(name="vv", bufs=2))
        tmp_pool = ep(tc.tile_pool(name="tmp", bufs=4))
        pt_pool = ep(tc.tile_pool(name="pt", bufs=4))
        rec_pool = ep(tc.tile_pool(name="rec", bufs=2))
        rb_pool = ep(tc.tile_pool(name="rb", bufs=2))
        zt_pool = ep(tc.tile_pool(name="zt", bufs=2))
        ot_pool = ep(tc.tile_pool(name="ot", bufs=3))

        ps_a = ep(tc.tile_pool(name="ps_a", bufs=2, space="PSUM"))  # proj / bcast
        ps_s = ep(tc.tile_pool(name="ps_s", bufs=2, space="PSUM"))  # scores
        ps_z = ep(tc.tile_pool(name="ps_z", bufs=2, space="PSUM"))  # z accum
        ps_o = ep(tc.tile_pool(name="ps_o", bufs=1, space="PSUM"))  # out proj

        # ---- constants / weights (loaded once) ----
        cos_sb = const.tile([DH, S], F32)
        nc.scalar.dma_start(out=cos_sb[:], in_=dram["cosc"][:])
        sin_sb = const.tile([DH, S], F32)
        nc.scalar.dma_start(out=sin_sb[:], in_=dram["sinc"][:])
        mask_sb = const.tile([128, 1024], BF16)
        nc.scalar.dma_start(out=mask_sb[:], in_=dram["maskt"][:])
        bq_sb = const.tile([DH, PAIRS], F32)
        nc.scalar.dma_start(out=bq_sb[:], in_=dram["bq"][:])
        bk_sb = const.tile([DH, PAIRS], F32)
        nc.scalar.dma_start(out=bk_sb[:], in_=dram["bk"][:])
        bqf_sb = const.tile([DH, PAIRS], F32)
        nc.scalar.dma_start(out=bqf_sb[:], in_=dram["bqf"][:])
        bkf_sb = const.tile([DH, PAIRS], F32)
        nc.scalar.dma_start(out=bkf_sb[:], in_=dram["bkf"][:])
        ones_sb = const.tile([128, DH], F32)
        nc.vector.memset(ones_sb[:], 1.0)
        w_sb = {}
        for t in ("wq", "wk", "wv"):
            w = const.tile([128, PAIRS * MC * DH], BF16, tag=f"w_{t}")
            nc.scalar.dma_start(out=w[:], in_=dram[t][:])
            w_sb[t] = w
        wo_sb = const.tile([DH, PAIRS * M], BF16)
        nc.scalar.dma_start(out=wo_sb[:], in_=dram["wo"][:])

        for p in range(PAIRS):
            # ---- load xT tiles via DMA transpose ----
            xt = {}
            for t in ("xq", "xk", "xv"):
                for mc in range(MC):
                    xtile = xt_pool.tile([128, S], BF16, tag="xt")
                    nc.sync.dma_start(out=xtile[:], in_=dram[t][p, mc], transpose=True)
                    xt[(t, mc)] = xtile

            # ---- q/k projections + rotary ----
            qT = qk_pool.tile([DH, S], BF16, tag="qT")
            kT = qk_pool.tile([DH, S], BF16, tag="kT")
            for t, dst, cs_all, sn_all, b_all, bf_all in (
                ("xq", qT, cos_sb, sin_sb, bq_sb, bqf_sb),
                ("xk", kT, cos_sb, sin_sb, bk_sb, bkf_sb),
            ):
                wname = "wq" if t == "xq" else "wk"
                for qc in range(QC):
                    c0 = qc * QCHUNK
                    ps = ps_a.tile([DH, QCHUNK], F32, tag="ps_a")
                    for mc in range(MC):
                        nc.tensor.matmul(
                            ps[:],
                            w_sb[wname][:, (p * MC + mc) * DH : (p * MC + mc + 1) * DH],
                            xt[(t, mc)][:, c0 : c0 + QCHUNK],
                            start=(mc == 0),
                            stop=(mc == MC - 1),
                        )
                    # rotary + bias + bf16 cast on eviction
                    tcos = tmp_pool.tile([DH, QCHUNK], F32, tag="tcos")
                    nc.vector.scalar_tensor_tensor(
                        tcos[:], ps[:], b_all[:, p : p + 1],
                        cs_all[:, c0 : c0 + QCHUNK], op0=ADD, op1=MUL,
                    )
                    tflip = tmp_pool.tile([DH, QCHUNK], F32, tag="tflip")
                    nc.vector.scalar_tensor_tensor(
                        tflip[0:32, :], ps[32:64, :], bf_all[0:32, p : p + 1],
                        sn_all[0:32, c0 : c0 + QCHUNK], op0=ADD, op1=MUL,
                    )
                    nc.vector.scalar_tensor_tensor(
                        tflip[32:64, :], ps[0:32, :], bf_all[32:64, p : p + 1],
                        sn_all[32:64, c0 : c0 + QCHUNK], op0=ADD, op1=MUL,
                    )
                    nc.vector.tensor_add(dst[:, c0 : c0 + QCHUNK], tcos[:], tflip[:])

            # ---- v projection (natural layout, with ones column) ----
            vv = vv_pool.tile([128, 16 * (DH + 1)], BF16, tag="vv")
            nc.vector.memset(
                vv[:].rearrange("p (t c) -> p t c", c=DH + 1)[:, :, DH : DH + 1], 1.0
            )
            for sc in range(16):
                psv = ps_a.tile([128, DH], F32, tag="ps_a")
                for mc in range(MC):
                    nc.tensor.matmul(
                        psv[:],
                        xt[("xv", mc)][:, sc * 128 : (sc + 1) * 128],
                        w_sb["wv"][:, (p * MC + mc) * DH : (p * MC + mc + 1) * DH],
                        start=(mc == 0),
                        stop=(mc == MC - 1),
                    )
                nc.vector.tensor_copy(vv[:, sc * (DH + 1) : sc * (DH + 1) + DH], psv[:])

            # ---- attention, one 512-wide q chunk at a time ----
            for qc in range(QC):
                q0 = qc * QCHUNK
                nkb = (qc + 1) * 4
                zps = ps_z.tile([DH + 1, QCHUNK], F32, tag="ps_z")
                for kb in range(nkb):
                    sps = ps_s.tile([128, QCHUNK], F32, tag="ps_s")
                    nc.tensor.matmul(
                        sps[:],
                        kT[:, kb * 128 : (kb + 1) * 128],
                        qT[:, q0 : q0 + QCHUNK],
                        start=True,
                        stop=True,
                    )
                    pt = pt_pool.tile([128, QCHUNK], BF16, tag="pt")
                    nc.scalar.activation(pt[:], sps[:], EXP)
                    if kb >= qc * 4:  # diagonal block: 0/1 causal mask
                        d = kb * 128 - q0
                        nc.vector.tensor_mul(
                            pt[:], pt[:], mask_sb[:, 512 - d : 1024 - d]
                        )
                    nc.tensor.matmul(
                        zps[:],
                        vv[:, kb * (DH + 1) : (kb + 1) * (DH + 1)],
                        pt[:],
                        start=(kb == 0),
                        stop=(kb == nkb - 1),
                    )
                # 1/rowsum, broadcast over the 64 d-partitions via K=1 matmul
                rec = rec_pool.tile([DH + 1, QCHUNK], F32, tag="rec")
                nc.vector.reciprocal(rec[DH : DH + 1, :], zps[DH : DH + 1, :])
                bps = ps_a.tile([DH, QCHUNK], F32, tag="ps_a")
                nc.tensor.matmul(
                    bps[:], ones_sb[DH : DH + 1, :], rec[DH : DH + 1, :],
                    start=True, stop=True,
                )
                rb = rb_pool.tile([DH, QCHUNK], F32, tag="rb")
                nc.scalar.copy(rb[:], bps[:])
                zt = zt_pool.tile([DH, QCHUNK], BF16, tag="zt")
                nc.vector.tensor_mul(zt[:], zps[0:DH, :], rb[:])

                # ---- out projection for the 4 q-blocks of this chunk ----
                for qb in range(4):
                    ops = ps_o.tile([128, M], F32, tag="ps_o")
                    zblk = zt[:, qb * 128 : (qb + 1) * 128]
                    nc.tensor.matmul(
                        ops[:, 0:512], zblk, wo_sb[:, p * M : p * M + 512],
                        start=True, stop=True,
                    )
                    nc.tensor.matmul(
                        ops[:, 512:768], zblk, wo_sb[:, p * M + 512 : p * M + M],
                        start=True, stop=True,
                    )
                    ot = ot_pool.tile([128, M], BF16, tag="ot")
                    nc.scalar.copy(ot[:], ops[:])
                    r0 = q0 + qb * 128
                    nc.scalar.dma_start(out=out_d[p, r0 : r0 + 128, :], in_=ot[:])

    nc.compile()
    return nc


_NC = None


def _get_nc():
    global _NC
    if _NC is None:
        _NC = build_program()
    return _NC


def _rotary_tables():
    pos = np.arange(S, dtype=np.float64)
    dim = np.arange(DH // 2, dtype=np.float64)
    freq = ROTARY_BASE ** (dim / (DH // 2))
    freq = np.concatenate([freq, freq])
    ang = pos[:, None] / freq[None, :]  # [S, 64]
    cosT = np.cos(ang).T  # [64, S]
    sinT = np.sin(ang).T
    sin_signed = np.concatenate([-sinT[: DH // 2], sinT[DH // 2 :]], axis=0)
    return (GS * cosT).astype(np.float32), (GS * sin_signed).astype(np.float32)


def host_inputs(inputs):
    """Slice/cast the full problem inputs into 8 per-core in_maps."""
    bf = ml_dtypes.bfloat16
    xs = {}
    for key, name in (
        ("normalized_resid_pre_q", "xq"),
        ("normalized_resid_pre_k", "xk"),
        ("normalized_resid_pre_v", "xv"),
    ):
        x = np.asarray(inputs[key])  # [B, S, H, M] f32
        # -> [B*H, MC, S, 128] bf16, pair-major
        x = np.ascontiguousarray(x.transpose(0, 2, 1, 3)).astype(bf)  # [B,H,S,M]
        x = x.reshape(B * H, S, MC, 128).transpose(0, 2, 1, 3)  # [24, MC, S, 128]
        xs[name] = np.ascontiguousarray(x)

    wq = np.asarray(inputs["W_Q"]).astype(bf)  # [H, M, DH]
    wk = np.asarray(inputs["W_K"]).astype(bf)
    wv = np.asarray(inputs["W_V"]).astype(bf)
    wo = np.asarray(inputs["W_O"]).astype(bf)  # [H, DH, M]
    bq = np.asarray(inputs["b_Q"]).astype(np.float32)  # [H, DH]
    bk = np.asarray(inputs["b_K"]).astype(np.float32)

    cosc, sinc = _rotary_tables()
    maskt = (
        np.arange(1024, dtype=np.int32)[None, :]
        >= np.arange(128, dtype=np.int32)[:, None] + 512
    ).astype(bf)

    in_maps = []
    for c in range(N_CORES):
        pairs = [(3 * c + i) for i in range(PAIRS)]
        heads = [p % H for p in pairs]
        def pack_w(w):  # [3 heads, 768, 64] -> [128, 3*6*64]
            return np.ascontiguousarray(
                w.reshape(PAIRS, MC, 128, DH).transpose(2, 0, 1, 3).reshape(128, -1)
            )

        m = {
            "xq": xs["xq"][pairs[0] : pairs[0] + PAIRS],
            "xk": xs["xk"][pairs[0] : pairs[0] + PAIRS],
            "xv": xs["xv"][pairs[0] : pairs[0] + PAIRS],
            "wq": pack_w(wq[heads]),
            "wk": pack_w(wk[heads]),
            "wv": pack_w(wv[heads]),
            "wo": np.ascontiguousarray(
                wo[heads].transpose(1, 0, 2).reshape(DH, PAIRS * M)
            ),
            "cosc": cosc,
            "sinc": sinc,
            "maskt": maskt,
            "bq": np.ascontiguousarray(bq[heads].T),  # [DH, PAIRS]
            "bk": np.ascontiguousarray(bk[heads].T),
            "bqf": np.ascontiguousarray(
                np.concatenate([bq[heads].T[32:], bq[heads].T[:32]], axis=0)
            ),
            "bkf": np.ascontiguousarray(
                np.concatenate([bk[heads].T[32:], bk[heads].T[:32]], axis=0)
            ),
        }
        in_maps.append(m)
    return in_maps


def assemble_output(results, inputs):
    """[core]["out"] [PAIRS, S, M] bf16 -> [B, S, H, M] f32 (+ exact host biases)."""
    outs = np.stack([np.asarray(r["out"], dtype=np.float32) for r in results])
    out = outs.reshape(B, H, S, M).transpose(0, 2, 1, 3)  # pair p = b*H + h
    bo = np.asarray(inputs["b_O"], dtype=np.float64) / H  # [M]
    bv = np.asarray(inputs["b_V"], dtype=np.float64)  # [H, DH]
    wo = np.asarray(inputs["W_O"], dtype=np.float64)  # [H, DH, M]
    corr = np.einsum("hd,hdm->hm", bv, wo) + bo[None, :]  # [H, M]
    if np.any(corr):
        out = out + corr[None, None].astype(np.float32)
    return np.ascontiguousarray(out.astype(np.float32))


def kernel(**inputs):
    nc = _get_nc()
    in_maps = host_inputs(inputs)
    res = run_bass_kernel_spmd(
        nc, in_maps, core_ids=list(range(N_CORES)), trace=TRACE
    )
    if TRACE and res.exec_time_ns is not None:
        kernel.last_exec_time_ns = res.exec_time_ns
    return assemble_output(res.results, inputs)


kernel.last_exec_time_ns = None


# revision 14
# speedup vs baseline: 1.1208x; 1.1208x over previous
"""Trainium2 Bass kernel for nn_Attention_24404004176269.

Rotary causal attention with per-head inputs/weights:
  x_{q,k,v}: [B=2, S=2048, H=12, M=768], W_{Q,K,V}: [H, 768, 64], W_O: [H, 64, 768]
  out[b,s,h,:] = softmax(causal(rot(q) rot(k)^T / 8)) @ v @ W_O[h] (+ biases)

Sharding: the 24 (b, h) pairs are fully independent -> 3 pairs per core on 8 cores.

Per-core plan (all compute in bf16 with fp32 PSUM accumulation):
  - host casts inputs to bf16 and pre-chunks x as [pair, mc, S, 128] so the
    xbar DMA-transpose loads xT tiles [128m, S] straight from DRAM
  - qT/kT [64, S] = W^T @ xT, rotary (+ bias, + 1/sqrt(8) scale folded into
    the cos/sin tables) applied during PSUM eviction
  - v [S, 64] = xT^T @ W_V, stored as [128k, 65] tiles with a ones column so
    the z matmul also produces softmax row-sums
  - scores computed transposed: sT [128k, 512q] = kT_blk^T @ qT_chunk, so
    exp(sT) is directly the rhs of zT [65, 512q] += v_blk^T-style accumulation
  - causal masking: fully-masked key blocks are skipped; diagonal blocks get a
    0/1 multiplicative mask after exp (exact, matches exp(-1e5-max)==0)
  - normalization 1/rowsum broadcast across the 64 d-partitions via a K=1
    matmul, multiplied into zT during eviction
  - out [128q, 768] = zT_blk^T @ W_O, evicted bf16, DMA'd out
  - b_V and b_O are folded in exactly on the host: softmax rows sum to 1, so
    z = P(v + b_V) = Pv + b_V, giving out += b_V @ W_O + b_O/H per head.
"""

import sys

for _p in ("/opt/trn_rl_repo",):
    if _p not in sys.path:
        sys.path.insert(0, _p)

import contextlib

import ml_dtypes
import numpy as np

import concourse.bass as bass
import concourse.tile as tile
from concourse import bacc, mybir
from concourse.bass_utils import run_bass_kernel_spmd

B, S, H, M, DH = 2, 2048, 12, 768, 64
N_CORES = 8
PAIRS = (B * H) // N_CORES  # 3 (b, h) pairs per core
MC = M // 128  # 6 contraction chunks
QC = 4  # q chunks of 512
QCHUNK = 512
ROTARY_BASE = 10000.0
GS = float(np.sqrt(1.0 / np.sqrt(float(DH))))  # sqrt(1/8), folded into q AND k

BF16 = mybir.dt.bfloat16
F32 = mybir.dt.float32
MUL = mybir.AluOpType.mult
ADD = mybir.AluOpType.add
EXP = mybir.ActivationFunctionType.Exp

TRACE = False  # test.py can flip this for neuron-profile timing


def build_program():
    """Build the per-core Bass program (identical on all cores, SPMD by data)."""
    nc = bacc.Bacc(None, target_bir_lowering=False, debug=False, num_devices=N_CORES)

    dram = {}
    for t in ("xq", "xk", "xv"):
        dram[t] = nc.dram_tensor(t, [PAIRS, MC, S, 128], BF16, kind="ExternalInput").ap()
    for t in ("wq", "wk", "wv"):
        # host pre-packed: [128, PAIRS*MC*DH], column block (p*MC+mc)*DH is
        # W[head_p][mc*128:(mc+1)*128, :]
        dram[t] = nc.dram_tensor(t, [128, PAIRS * MC * DH], BF16, kind="ExternalInput").ap()
    dram["wo"] = nc.dram_tensor("wo", [DH, PAIRS * M], BF16, kind="ExternalInput").ap()
    dram["cosc"] = nc.dram_tensor("cosc", [DH, S], F32, kind="ExternalInput").ap()
    dram["sinc"] = nc.dram_tensor("sinc", [DH, S], F32, kind="ExternalInput").ap()
    dram["maskt"] = nc.dram_tensor("maskt", [128, 1024], BF16, kind="ExternalInput").ap()
    dram["bq"] = nc.dram_tensor("bq", [DH, PAIRS], F32, kind="ExternalInput").ap()
    dram["bk"] = nc.dram_tensor("bk", [DH, PAIRS], F32, kind="ExternalInput").ap()
    # partition-flipped copies (rows 0:32 <-> 32:64) so the rotary flip term's
    # scalar operand shares its base partition with the sin table slice
    dram["bqf"] = nc.dram_tensor("bqf", [DH, PAIRS], F32, kind="ExternalInput").ap()
    dram["bkf"] = nc.dram_tensor("bkf", [DH, PAIRS], F32, kind="ExternalInput").ap()
    out_d = nc.dram_tensor("out", [PAIRS, S, M], BF16, kind="ExternalOutput").ap()

    with tile.TileContext(nc) as tc, contextlib.ExitStack() as ctx:
        ep = ctx.enter_context

        const = ep(tc.tile_pool(name="const", bufs=1))
        xt_pool = ep(tc.tile_pool(name="xt", bufs=24))
        qk_pool = ep(tc.tile_pool(name="qk", bufs=4))
        vv_pool = ep(tc.tile_pool(name="vv", bufs=2))
        tmp_pool = ep(tc.tile_pool(name="tmp", bufs=4))
        pt_pool = ep(tc.tile_pool(name="pt", bufs=4))
        rec_pool = ep(tc.tile_pool(name="rec", bufs=2))
        rb_pool = ep(tc.tile_pool(name="rb", bufs=2))
        zt_pool = ep(tc.tile_pool(name="zt", bufs=2))
        ot_pool = ep(tc.tile_pool(name="ot", bufs=3))

        ps_a = ep(tc.tile_pool(name="ps_a", bufs=2, space="PSUM"))  # proj / bcast
        ps_s = ep(tc.tile_pool(name="ps_s", bufs=2, space="PSUM"))  # scores
        ps_z = ep(tc.tile_pool(name="ps_z", bufs=2, space="PSUM"))  # z accum
        ps_o = ep(tc.tile_pool(name="ps_o", bufs=1, space="PSUM"))  # out proj

        # ---- constants / weights (loaded once) ----
        cos_sb = const.tile([DH, S], F32)
        nc.scalar.dma_start(out=cos_sb[:], in_=dram["cosc"][:])
        sin_sb = const.tile([DH, S], F32)
        nc.scalar.dma_start(out=sin_sb[:], in_=dram["sinc"][:])
        mask_sb = const.tile([128, 1024], BF16)
        nc.scalar.dma_start(out=mask_sb[:], in_=dram["maskt"][:])
        bq_sb = const.tile([DH, PAIRS], F32)
        nc.scalar.dma_start(out=bq_sb[:], in_=dram["bq"][:])
        bk_sb = const.tile([DH, PAIRS], F32)
        nc.scalar.dma_start(out=bk_sb[:], in_=dram["bk"][:])
        bqf_sb = const.tile([DH, PAIRS], F32)
        nc.scalar.dma_start(out=bqf_sb[:], in_=dram["bqf"][:])
        bkf_sb = const.tile([DH, PAIRS], F32)
        nc.scalar.dma_start(out=bkf_sb[:], in_=dram["bkf"][:])

        w_sb = {}
        for t in ("wq", "wk", "wv"):
            w = const.tile([128, PAIRS * MC * DH], BF16, tag=f"w_{t}")
            nc.scalar.dma_start(out=w[:], in_=dram[t][:])
            w_sb[t] = w
        wo_sb = const.tile([DH, PAIRS * M], BF16)
        nc.scalar.dma_start(out=wo_sb[:], in_=dram["wo"][:])

        for p in range(PAIRS):
            # ---- load xT tiles via DMA transpose ----
            xt = {}
            for t in ("xq", "xk", "xv"):
                for mc in range(MC):
                    xtile = xt_pool.tile([128, S], BF16, tag="xt")
                    nc.sync.dma_start(out=xtile[:], in_=dram[t][p, mc], transpose=True)
                    xt[(t, mc)] = xtile

            # ---- q/k projections + rotary ----
            qT = qk_pool.tile([DH, S], BF16, tag="qT")
            kT = qk_pool.tile([DH, S], BF16, tag="kT")
            for t, dst, cs_all, sn_all, b_all, bf_all in (
                ("xq", qT, cos_sb, sin_sb, bq_sb, bqf_sb),
                ("xk", kT, cos_sb, sin_sb, bk_sb, bkf_sb),
            ):
                wname = "wq" if t == "xq" else "wk"
                for qc in range(QC):
                    c0 = qc * QCHUNK
                    ps = ps_a.tile([DH, QCHUNK], F32, tag="ps_a")
                    for mc in range(MC):
                        nc.tensor.matmul(
                            ps[:],
                            w_sb[wname][:, (p * MC + mc) * DH : (p * MC + mc + 1) * DH],
                            xt[(t, mc)][:, c0 : c0 + QCHUNK],
                            start=(mc == 0),
                            stop=(mc == MC - 1),
                        )
                    # rotary + bias + bf16 cast on eviction
                    tcos = tmp_pool.tile([DH, QCHUNK], F32, tag="tcos")
                    nc.vector.scalar_tensor_tensor(
                        tcos[:], ps[:], b_all[:, p : p + 1],
                        cs_all[:, c0 : c0 + QCHUNK], op0=ADD, op1=MUL,
                    )
                    tflip = tmp_pool.tile([DH, QCHUNK], F32, tag="tflip")
                    nc.vector.scalar_tensor_tensor(
                        tflip[0:32, :], ps[32:64, :], bf_all[0:32, p : p + 1],
                        sn_all[0:32, c0 : c0 + QCHUNK], op0=ADD, op1=MUL,
                    )
                    nc.vector.scalar_tensor_tensor(
                        tflip[32:64, :], ps[0:32, :], bf_all[32:64, p : p + 1],
                        sn_all[32:64, c0 : c0 + QCHUNK], op0=ADD, op1=MUL,
                    )
                    nc.vector.tensor_add(dst[:, c0 : c0 + QCHUNK], tcos[:], tflip[:])

            # ---- v projection (natural layout, with ones column) ----
            vv = vv_pool.tile([128, 16 * (DH + 1)], BF16, tag="vv")
            nc.vector.memset(
                vv[:].rearrange("p (t c) -> p t c", c=DH + 1)[:, :, DH : DH + 1], 1.0
            )
            for sc in range(16):
                psv = ps_a.tile([128, DH], F32, tag="ps_a")
                for mc in range(MC):
                    nc.tensor.matmul(
                        psv[:],
                        xt[("xv", mc)][:, sc * 128 : (sc + 1) * 128],
                        w_sb["wv"][:, (p * MC + mc) * DH : (p * MC + mc + 1) * DH],
                        start=(mc == 0),
                        stop=(mc == MC - 1),
                    )
                nc.vector.tensor_copy(vv[:, sc * (DH + 1) : sc * (DH + 1) + DH], psv[:])

            # ---- attention, one 512-wide q chunk at a time ----
            for qc in range(QC):
                q0 = qc * QCHUNK
                nkb = (qc + 1) * 4
                zps = ps_z.tile([DH + 1, QCHUNK], F32, tag="ps_z")

                # scores are emitted one kb ahead of the z matmuls so the PE
                # stream never stalls waiting for ACT's exp
                def score(kb):
                    sps = ps_s.tile([128, QCHUNK], F32, tag="ps_s")
                    nc.tensor.matmul(
                        sps[:],
                        kT[:, kb * 128 : (kb + 1) * 128],
                        qT[:, q0 : q0 + QCHUNK],
                        start=True,
                        stop=True,
                    )
                    pt = pt_pool.tile([128, QCHUNK], BF16, tag="pt")
                    nc.scalar.activation(pt[:], sps[:], EXP)
                    if kb >= qc * 4:  # diagonal block: 0/1 causal mask
                        d = kb * 128 - q0
                        nc.vector.tensor_mul(
                            pt[:], pt[:], mask_sb[:, 512 - d : 1024 - d]
                        )
                    return pt

                pts = {0: score(0)}
                for kb in range(nkb):
                    if kb + 1 < nkb:
                        pts[kb + 1] = score(kb + 1)
                    nc.tensor.matmul(
                        zps[:],
                        vv[:, kb * (DH + 1) : (kb + 1) * (DH + 1)],
                        pts.pop(kb)[:],
                        start=(kb == 0),
                        stop=(kb == nkb - 1),
                    )
                # 1/rowsum, broadcast over the 64 d-partitions on GpSimd
                rec = rec_pool.tile([1, QCHUNK], F32, tag="rec")
                nc.vector.reciprocal(rec[:], zps[DH : DH + 1, :])
                rb = rb_pool.tile([DH, QCHUNK], F32, tag="rb")
                nc.gpsimd.partition_broadcast(rb[:], rec[:])
                zt = zt_pool.tile([DH, QCHUNK], BF16, tag="zt")
                nc.vector.tensor_mul(zt[:], zps[0:DH, :], rb[:])

                # ---- out projection for the 4 q-blocks of this chunk ----
                for qb in range(4):
                    ops = ps_o.tile([128, M], F32, tag="ps_o")
                    zblk = zt[:, qb * 128 : (qb + 1) * 128]
                    nc.tensor.matmul(
                        ops[:, 0:512], zblk, wo_sb[:, p * M : p * M + 512],
                        start=True, stop=True,
                    )
                    nc.tensor.matmul(
                        ops[:, 512:768], zblk, wo_sb[:, p * M + 512 : p * M + M],
                        start=True, stop=True,
                    )
                    ot = ot_pool.tile([128, M], BF16, tag="ot")
                    nc.scalar.copy(ot[:], ops[:])
                    r0 = q0 + qb * 128
                    nc.scalar.dma_start(out=out_d[p, r0 : r0 + 128, :], in_=ot[:])

    nc.compile()
    return nc


_NC = None


def _get_nc():
    global _NC
    if _NC is None:
        _NC = build_program()
    return _NC


def _rotary_tables():
    pos = np.arange(S, dtype=np.float64)
    dim = np.arange(DH // 2, dtype=np.float64)
    freq = ROTARY_BASE ** (dim / (DH // 2))
    freq = np.concatenate([freq, freq])
    ang = pos[:, None] / freq[None, :]  # [S, 64]
    cosT = np.cos(ang).T  # [64, S]
    sinT = np.sin(ang).T
    sin_signed = np.concatenate([-sinT[: DH // 2], sinT[DH // 2 :]], axis=0)
    return (GS * cosT).astype(np.float32), (GS * sin_signed).astype(np.float32)


def host_inputs(inputs):
    """Slice/cast the full problem inputs into 8 per-core in_maps."""
    bf = ml_dtypes.bfloat16
    xs = {}
    for key, name in (
        ("normalized_resid_pre_q", "xq"),
        ("normalized_resid_pre_k", "xk"),
        ("normalized_resid_pre_v", "xv"),
    ):
        x = np.asarray(inputs[key])  # [B, S, H, M] f32
        # -> [B*H, MC, S, 128] bf16, pair-major
        x = np.ascontiguousarray(x.transpose(0, 2, 1, 3)).astype(bf)  # [B,H,S,M]
        x = x.reshape(B * H, S, MC, 128).transpose(0, 2, 1, 3)  # [24, MC, S, 128]
        xs[name] = np.ascontiguousarray(x)

    wq = np.asarray(inputs["W_Q"]).astype(bf)  # [H, M, DH]
    wk = np.asarray(inputs["W_K"]).astype(bf)
    wv = np.asarray(inputs["W_V"]).astype(bf)
    wo = np.asarray(inputs["W_O"]).astype(bf)  # [H, DH, M]
    bq = np.asarray(inputs["b_Q"]).astype(np.float32)  # [H, DH]
    bk = np.asarray(inputs["b_K"]).astype(np.float32)

    cosc, sinc = _rotary_tables()
    maskt = (
        np.arange(1024, dtype=np.int32)[None, :]
        >= np.arange(128, dtype=np.int32)[:, None] + 512
    ).astype(bf)

    in_maps = []
    for c in range(N_CORES):
        pairs = [(3 * c + i) for i in range(PAIRS)]
        heads = [p % H for p in pairs]
        def pack_w(w):  # [3 heads, 768, 64] -> [128, 3*6*64]
            return np.ascontiguousarray(
                w.reshape(PAIRS, MC, 128, DH).transpose(2, 0, 1, 3).reshape(128, -1)
            )

        m = {
            "xq": xs["xq"][pairs[0] : pairs[0] + PAIRS],
            "xk": xs["xk"][pairs[0] : pairs[0] + PAIRS],
            "xv": xs["xv"][pairs[0] : pairs[0] + PAIRS],
            "wq": pack_w(wq[heads]),
            "wk": pack_w(wk[heads]),
            "wv": pack_w(wv[heads]),
            "wo": np.ascontiguousarray(
                wo[heads].transpose(1, 0, 2).reshape(DH, PAIRS * M)
            ),
            "cosc": cosc,
            "sinc": sinc,
            "maskt": maskt,
            "bq": np.ascontiguousarray(bq[heads].T),  # [DH, PAIRS]
            "bk": np.ascontiguousarray(bk[heads].T),
            "bqf": np.ascontiguousarray(
                np.concatenate([bq[heads].T[32:], bq[heads].T[:32]], axis=0)
            ),
            "bkf": np.ascontiguousarray(
                np.concatenate([bk[heads].T[32:], bk[heads].T[:32]], axis=0)
            ),
        }
        in_maps.append(m)
    return in_maps


def assemble_output(results, inputs):
    """[core]["out"] [PAIRS, S, M] bf16 -> [B, S, H, M] f32 (+ exact host biases)."""
    outs = np.stack([np.asarray(r["out"], dtype=np.float32) for r in results])
    out = outs.reshape(B, H, S, M).transpose(0, 2, 1, 3)  # pair p = b*H + h
    bo = np.asarray(inputs["b_O"], dtype=np.float64) / H  # [M]
    bv = np.asarray(inputs["b_V"], dtype=np.float64)  # [H, DH]
    wo = np.asarray(inputs["W_O"], dtype=np.float64)  # [H, DH, M]
    corr = np.einsum("hd,hdm->hm", bv, wo) + bo[None, :]  # [H, M]
    if np.any(corr):
        out = out + corr[None, None].astype(np.float32)
    return np.ascontiguousarray(out.astype(np.float32))


def kernel(**inputs):
    nc = _get_nc()
    in_maps = host_inputs(inputs)
    res = run_bass_kernel_spmd(
        nc, in_maps, core_ids=list(range(N_CORES)), trace=TRACE
    )
    if TRACE and res.exec_time_ns is not None:
        kernel.last_exec_time_ns = res.exec_time_ns
    return assemble_output(res.results, inputs)


kernel.last_exec_time_ns = None
